# revision 14
# baseline (speedup 1.0000x reference)
"""Butterfly sparse-attention MLP kernel for 8 Trainium2 NeuronCores.

Computation (from the reference):
    attn = (w1.T @ w2.T) * sparse_mask          # [4096 s, 4096 t]
    y    = gelu(x @ attn + b2)                  # [8, 768, 4096]

sparse_mask has ~140k nonzeros in 6x6 blocks on five bands (offsets
0/+-64/+-128): attn[s, t] == 0 whenever |s - t| > 133.  Computing attn is
only ~1.2 GFLOP (0.6% of the total work), so the host computes the banded
attn during input prep via batched 6x4096x6 matmuls; the device kernel is
just the 206-GFLOP banded  y = gelu(x @ attn + b2).

Sharding: 4-way tensor-parallel over t (blocks of 1024) x 2-way data
parallel over rows n (blocks of 3072).  Each core loads an 1408-row
s-window of x^T (its 1024 t-block + 133-margin band, 11 chunks of 128),
the banded attn tiles for its block (1 MB), and streams
  yT[q] = gelu(attn[band(q)].T @ x_window + b2)     per 128-t subtile q,
contracting 4 s-chunks of 128 per subtile (band width 394 <= 512).

DMA queues dispatch ~1 descriptor (one SBUF partition row) per ~50 ns, so
every [128, W] transfer costs ~6.4 us of queue time until W reaches ~18 KB
rows (the HBM byte limit).  x travels as 24 KB-row quad-chunks, attn as one
8 KB-row transfer on the otherwise idle vector queue, y as 12 KB-row pair
stores; the final subtile is stored as four 32-partition slices on four
queues to cut the drain tail to ~2 us.  Prefetch order is gated so the
first quad (subtile 0's whole band) owns the full HBM bandwidth.
"""

import numpy as np

B, T, D = 8, 768, 4096
N = B * T              # 6144 rows of x
NCORES = 8
CT, CN = 4, 2          # t-split x n-split core grid
TB = D // CT           # 1024 t-columns per core
NB = N // CN           # 3072 x-rows per core
P = 128
NQ = TB // P           # 8 t-subtiles per core
MARGIN = 133           # band half-width of attn
NCH = 11               # s-chunks in the x window (128*11 = 1408 >= 1024+2*133)
BANDCH = 4             # s-chunks feeding one t-subtile (394-wide band)
MMN = 512              # moving-operand free-dim cap per matmul
NH = NB // MMN         # 6 n-pieces per core
PADT = MARGIN          # top padding of the s axis
PADROWS = D + P * (NCH - NQ) + PADT  # 4480: fits every core's window

_NC = None


def _build_module():
    from concourse import bacc, bass, mybir, tile

    f32 = mybir.dt.float32
    f16 = mybir.dt.float16
    PSUM = bass.MemorySpace.PSUM

    nc = bacc.Bacc("TRN2", target_bir_lowering=False, debug=False)
    # chunk 0-3 quad, split by n-columns: A = cols 0-2048, B = cols 2048-3072
    xqa_d = nc.declare_dram_parameter("xqa_s", [P, 4 * 2048], f16,
                                      isOutput=False)
    xqb_d = nc.declare_dram_parameter("xqb_s", [P, 4 * 1024], f16,
                                      isOutput=False)
    attn_d = nc.declare_dram_parameter(
        "attn_s", [P, NQ * BANDCH * P], f16, isOutput=False)
    xs_d = nc.declare_dram_parameter("xs_s", [6, P, NB], f16, isOutput=False)
    xt10_d = nc.declare_dram_parameter("xt10_s", [10, NB], f16, isOutput=False)
    b2_d = nc.declare_dram_parameter("b2c_s", [P, NQ], f32, isOutput=False)
    yp_d = nc.declare_dram_parameter("yp_s", [NQ // 2, P, 2 * NB], f16,
                                     isOutput=True)

    with tile.TileContext(nc) as tc:
        with (
            tc.tile_pool(name="const", bufs=1) as cpool,
            tc.tile_pool(name="xp", bufs=1) as xp,
            tc.tile_pool(name="psB", bufs=4, space=PSUM) as psB,
            tc.tile_pool(name="yp", bufs=4) as yp,
        ):
            b2_t = cpool.tile([P, NQ], f32)
            nc.gpsimd.dma_start(b2_t[:], b2_d[:])
            attn_t = cpool.tile([P, NQ * BANDCH * P], f16)
            nc.scalar.dma_start(attn_t[:], attn_d[:])
            # Head fill: quadA (16 KB rows) on sync + quadB (8 KB rows) on
            # gpsimd + attn on scalar land concurrently; everything else
            # queues behind them in FIFO order, so the first subtile's band
            # gets the whole per-core HBM bandwidth.
            xqa = xp.tile([P, 4 * 2048], f16, name="xqa")
            xqb = xp.tile([P, 4 * 1024], f16, name="xqb")
            nc.sync.dma_start(xqa[:], xqa_d[:])
            nc.gpsimd.dma_start(xqb[:], xqb_d[:])
            xs = []
            for k in range(6):  # chunks 4..9
                xt = xp.tile([P, NB], f16, name=f"xs{k}")
                eng = nc.scalar if k % 2 == 0 else nc.sync
                eng.dma_start(xt[:], xs_d[k])
                xs.append(xt)
            xt10 = xp.tile([10, NB], f16, name="xt10")
            nc.scalar.dma_start(xt10[:], xt10_d[:])

            def xpiece(c, h):
                """[128, 512] slice of x-window chunk c, n-piece h."""
                if c < 4:
                    if h < 4:
                        return xqa[:, c * 2048 + h * MMN:
                                   c * 2048 + (h + 1) * MMN]
                    return xqb[:, c * 1024 + (h - 4) * MMN:
                               c * 1024 + (h - 3) * MMN]
                if c < 10:
                    return xs[c - 4][:, h * MMN:(h + 1) * MMN]
                return xt10[:, h * MMN:(h + 1) * MMN]

            pair_t = None
            for q in range(NQ):
                if q % 2 == 0:
                    pair_t = yp.tile([P, 2 * NB], f16, name="y_pair",
                                     tag="y_pair")
                for hp in range(NH // 2):
                    y_ps = psB.tile([P, 2 * MMN], f32, name="y_ps", tag="y_ps")
                    for c in range(BANDCH):
                        # band tail: only 10 s-rows of chunk q+3 are in-band
                        kk = 10 if c == 3 else P
                        a_sl = attn_t[0:kk, (BANDCH * q + c) * P:
                                      (BANDCH * q + c + 1) * P]
                        for hh in range(2):
                            h = 2 * hp + hh
                            nc.tensor.matmul(
                                y_ps[:, hh * MMN:(hh + 1) * MMN],
                                a_sl,
                                xpiece(q + c, h)[0:kk, :],
                                start=(c == 0),
                                stop=(c == BANDCH - 1),
                            )
                    dst = pair_t[:, (q % 2) * NB + hp * 2 * MMN:
                                 (q % 2) * NB + (hp + 1) * 2 * MMN]
                    nc.scalar.activation(
                        dst,
                        y_ps[:],
                        mybir.ActivationFunctionType.Gelu,
                        bias=b2_t[:, q:q + 1],
                        scale=1.0,
                    )
                if q % 2 == 1:
                    # gpsimd SWDGE spreads paired stores over all 16 SDMA
                    # engines; the last pair goes on scalar to overlap.
                    eng = nc.scalar if q == NQ - 1 else nc.gpsimd
                    eng.dma_start(yp_d[q // 2], pair_t[:])

    nc.compile()
    nc.finalize()
    return nc


def _get_nc():
    global _NC
    if _NC is None:
        _NC = _build_module()
    return _NC


def _banded_attn(w1, w2, sparse_mask):
    """Host-side: (w1.T @ w2.T) * mask as a padded dense band, fp16.

    Mask nonzeros live in 6x6 blocks at (a + 6j + u, b + 6j + v) for shift
    pairs (a, b) in {(0,0), (64,0), (128,0), (0,64), (0,128)}; each block is
    one 6x4096x6 matmul, batched over j.  Returns [PADROWS, D] fp16 where
    row r corresponds to s = r - PADT.
    """
    PAD = 144  # covers max shift 128 + block overhang 6
    w1p = np.zeros((D, D + PAD), dtype=np.float32)
    w1p[:, :D] = w1
    w2p = np.zeros((D + PAD, D), dtype=np.float32)
    w2p[:D] = w2
    nblk = D // 6 + 1  # 683 blocks of 6 cover every in-range (s, t) block
    attn_pad = np.zeros((PADROWS, D + PAD), dtype=np.float32)
    jj = 6 * np.arange(nblk)
    for a, b in ((0, 0), (64, 0), (128, 0), (0, 64), (0, 128)):
        w1b = np.ascontiguousarray(
            w1p[:, a:a + 6 * nblk].reshape(D, nblk, 6).transpose(1, 2, 0))
        w2b = np.ascontiguousarray(
            w2p[b:b + 6 * nblk].reshape(nblk, 6, D))
        blocks = np.matmul(w1b, w2b.transpose(0, 2, 1))  # [nblk, 6 u, 6 v]
        rows = (PADT + a + jj[:, None] + np.arange(6)[None, :])  # [j, u]
        cols = (b + jj[:, None] + np.arange(6)[None, :])         # [j, v]
        attn_pad[rows[:, :, None], cols[:, None, :]] = blocks
    attn_pad = attn_pad[:, :D]
    attn_pad[:PADT] = 0.0
    attn_pad[PADT + D:] = 0.0
    attn_pad[PADT:PADT + D] *= np.asarray(sparse_mask, dtype=np.float32)
    return attn_pad.astype(np.float16)


def _band_ok(sparse_mask):
    """Every mask nonzero must sit in one of the five 6x6-block bands the
    host attn construction covers (and inside the kernel's s-window)."""
    s_idx, t_idx = np.nonzero(np.asarray(sparse_mask) != 0)
    if len(s_idx) == 0:
        return True
    da = s_idx - 6 * (t_idx // 6)   # type A: t-block-aligned
    db = t_idx - 6 * (s_idx // 6)   # type B: s-block-aligned
    allowed = np.concatenate([np.arange(k, k + 6) for k in (0, 64, 128)])
    return bool(np.all(np.isin(da, allowed) | np.isin(db, allowed)))


def prepare_in_maps(x, w1, w2, b2, sparse_mask):
    x = np.asarray(x, dtype=np.float32)
    w1 = np.asarray(w1, dtype=np.float32)
    w2 = np.asarray(w2, dtype=np.float32)
    b2 = np.asarray(b2, dtype=np.float32)

    attn16 = _banded_attn(w1, w2, sparse_mask)       # [PADROWS, D] fp16

    xT_pad = np.zeros((PADROWS, N), dtype=np.float16)
    xT_pad[PADT:PADT + D] = x.reshape(N, D).T

    in_maps = []
    for i in range(NCORES):
        it, inn = divmod(i, CN)
        t0 = it * TB
        n0 = inn * NB
        xw = np.ascontiguousarray(
            xT_pad[t0:t0 + NCH * P, n0:n0 + NB]).reshape(NCH, P, NB)
        at = np.empty((NQ, BANDCH, P, P), dtype=np.float16)
        for q in range(NQ):
            for c in range(BANDCH):
                r0 = t0 + P * (q + c)
                at[q, c] = attn16[r0:r0 + P, t0 + P * q:t0 + P * (q + 1)]
        in_maps.append({
            "xqa_s": np.ascontiguousarray(
                xw[:4, :, :2048].transpose(1, 0, 2).reshape(P, 4 * 2048)),
            "xqb_s": np.ascontiguousarray(
                xw[:4, :, 2048:].transpose(1, 0, 2).reshape(P, 4 * 1024)),
            "attn_s": np.ascontiguousarray(
                at.transpose(2, 0, 1, 3).reshape(P, NQ * BANDCH * P)),
            "xs_s": np.ascontiguousarray(xw[4:10]),
            "xt10_s": np.ascontiguousarray(xw[10, :10]),
            "b2c_s": np.ascontiguousarray(b2[t0:t0 + TB].reshape(NQ, P).T),
        })
    return in_maps


def assemble(results):
    out = np.empty((N, D), dtype=np.float32)
    for i in range(NCORES):
        it, inn = divmod(i, CN)
        t0 = it * TB
        n0 = inn * NB
        yp = results[i]["yp_s"]                      # [4, P, 2*NB]
        yT = np.empty((NQ, P, NB), dtype=np.float16)
        yT[0::2] = yp[:, :, :NB]
        yT[1::2] = yp[:, :, NB:]
        out[n0:n0 + NB, t0:t0 + TB] = (
            yT.transpose(2, 0, 1).reshape(NB, TB).astype(np.float32))
    return out.reshape(B, T, D)


def _reference_fallback(x, w1, w2, b2, sparse_mask):
    import jax
    import jax.numpy as jnp

    cpu = jax.devices("cpu")[0]
    with jax.default_device(cpu):
        attn = jnp.einsum("ds,td->st", jnp.asarray(w1), jnp.asarray(w2))
        attn = attn * jnp.asarray(sparse_mask)
        y = jnp.einsum("bds,st->bdt", jnp.asarray(x), attn) + jnp.asarray(b2)
        return np.asarray(jax.nn.gelu(y, approximate=False), dtype=np.float32)


def kernel(x, w1, w2, b2, sparse_mask):
    import time

    from concourse.bass_utils import run_bass_kernel_spmd

    if (np.shape(x) != (B, T, D) or np.shape(w1) != (D, D)
            or np.shape(w2) != (D, D) or np.shape(b2) != (D,)
            or np.shape(sparse_mask) != (D, D) or not _band_ok(sparse_mask)):
        return _reference_fallback(x, w1, w2, b2, sparse_mask)

    in_maps = prepare_in_maps(x, w1, w2, b2, sparse_mask)
    nc = _get_nc()
    last_err = None
    for attempt in range(3):
        try:
            res = run_bass_kernel_spmd(nc, in_maps, list(range(NCORES)))
            return assemble(res.results)
        except Exception as e:  # transient NRT/device errors: retry
            last_err = e
            time.sleep(2.0 * (attempt + 1))
    raise last_err


# revision 16
# speedup vs baseline: 1.2398x; 1.2398x over previous
"""Butterfly sparse-attention MLP kernel for 8 Trainium2 NeuronCores.

Computation (from the reference):
    attn = (w1.T @ w2.T) * sparse_mask          # [4096 s, 4096 t]
    y    = gelu(x @ attn + b2)                  # [8, 768, 4096]

sparse_mask has ~140k nonzeros in 6x6 blocks on five bands (offsets
0/+-64/+-128): attn[s, t] == 0 whenever |s - t| > 133.  Computing attn is
only ~1.2 GFLOP (0.6% of the total work), so the host computes the banded
attn during input prep via batched 6x4096x6 matmuls; the device kernel is
just the 206-GFLOP banded  y = gelu(x @ attn + b2).

Sharding: 4-way tensor-parallel over t (blocks of 1024) x 2-way data
parallel over rows n (blocks of 3072).  Each core loads an 1408-row
s-window of x^T (its 1024 t-block + 133-margin band, 11 chunks of 128),
the banded attn tiles for its block (1 MB), and streams
  yT[q] = gelu(attn[band(q)].T @ x_window + b2)     per 128-t subtile q,
contracting 4 s-chunks of 128 per subtile (band width 394 <= 512).

DMA queues dispatch ~1 descriptor (one SBUF partition row) per ~50 ns, so
every [128, W] transfer costs ~6.4 us of queue time until W reaches ~18 KB
rows (the HBM byte limit).  x travels as 24 KB-row quad-chunks, attn as one
8 KB-row transfer on the otherwise idle vector queue, y as 12 KB-row pair
stores; the final subtile is stored as four 32-partition slices on four
queues to cut the drain tail to ~2 us.  Prefetch order is gated so the
first quad (subtile 0's whole band) owns the full HBM bandwidth.
"""

import numpy as np

B, T, D = 8, 768, 4096
N = B * T              # 6144 rows of x
NCORES = 8
CT, CN = 4, 2          # t-split x n-split core grid
TB = D // CT           # 1024 t-columns per core
NB = N // CN           # 3072 x-rows per core
P = 128
NQ = TB // P           # 8 t-subtiles per core
MARGIN = 133           # band half-width of attn
NCH = 11               # s-chunks in the x window (128*11 = 1408 >= 1024+2*133)
BANDCH = 4             # s-chunks feeding one t-subtile (394-wide band)
MMN = 512              # moving-operand free-dim cap per matmul
NH = NB // MMN         # 6 n-pieces per core
PADT = MARGIN          # top padding of the s axis
PADROWS = D + P * (NCH - NQ) + PADT  # 4480: fits every core's window

_NC = None


def _build_module():
    from concourse import bacc, bass, mybir, tile

    f32 = mybir.dt.float32
    f16 = mybir.dt.float16
    PSUM = bass.MemorySpace.PSUM

    nc = bacc.Bacc("TRN2", target_bir_lowering=False, debug=False)
    xq_d = nc.declare_dram_parameter("xq_s", [P, 4 * NB], f16, isOutput=False)
    attn_d = nc.declare_dram_parameter(
        "attn_s", [P, NQ * BANDCH * P], f16, isOutput=False)
    xc4_d = nc.declare_dram_parameter("xc4_s", [P, NB], f16, isOutput=False)
    xs_d = nc.declare_dram_parameter("xs_s", [6, P, NB], f16, isOutput=False)
    b2_d = nc.declare_dram_parameter("b2c_s", [P, NQ], f32, isOutput=False)
    yp_d = nc.declare_dram_parameter("yp_s", [NQ // 2, P, 2 * NB], f16,
                                     isOutput=True)

    AQ = 2 * BANDCH * P  # attn columns for the first two subtiles

    with tile.TileContext(nc) as tc:
        with (
            tc.tile_pool(name="const", bufs=1) as cpool,
            tc.tile_pool(name="xp", bufs=1) as xp,
            tc.tile_pool(name="psB", bufs=4, space=PSUM) as psB,
            tc.tile_pool(name="yp", bufs=4) as yp,
        ):
            # Head fill (~4 MB gates the first matmul): chunk 0-3 quad
            # (24 KB rows) solo on sync, attn tiles for subtiles 0-1 on
            # scalar, b2 + chunk 4 on gpsimd.  Everything else queues
            # behind in FIFO order so the head owns the HBM bandwidth.
            attn_t = cpool.tile([P, NQ * BANDCH * P], f16)
            nc.scalar.dma_start(attn_t[:, :AQ], attn_d[:, :AQ])
            nc.scalar.dma_start(attn_t[:, AQ:], attn_d[:, AQ:])
            xq0 = xp.tile([P, 4 * NB], f16, name="xq0")
            nc.sync.dma_start(xq0[:], xq_d[:])
            b2_t = cpool.tile([P, NQ], f32)
            nc.gpsimd.dma_start(b2_t[:], b2_d[:])
            xc4 = xp.tile([P, NB], f16, name="xc4")
            nc.gpsimd.dma_start(xc4[:], xc4_d[:])
            xs = []
            for k in range(6):  # chunks 5..10 queue behind the quad on sync
                xt = xp.tile([P, NB], f16, name=f"xs{k}")
                nc.sync.dma_start(xt[:], xs_d[k])
                xs.append(xt)

            def xchunk(c):
                if c < 4:
                    return xq0[:, c * NB:(c + 1) * NB]
                if c == 4:
                    return xc4[:]
                return xs[c - 5][:]

            pair_t = None
            for q in range(NQ):
                if q % 2 == 0:
                    pair_t = yp.tile([P, 2 * NB], f16, name="y_pair",
                                     tag="y_pair")
                for hp in range(NH // 2):
                    y_ps = psB.tile([P, 2 * MMN], f32, name="y_ps", tag="y_ps")
                    for c in range(BANDCH):
                        a_sl = attn_t[:, (BANDCH * q + c) * P:
                                      (BANDCH * q + c + 1) * P]
                        xc = xchunk(q + c)
                        for hh in range(2):
                            h = 2 * hp + hh
                            nc.tensor.matmul(
                                y_ps[:, hh * MMN:(hh + 1) * MMN],
                                a_sl,
                                xc[:, h * MMN:(h + 1) * MMN],
                                start=(c == 0),
                                stop=(c == BANDCH - 1),
                            )
                    dst = pair_t[:, (q % 2) * NB + hp * 2 * MMN:
                                 (q % 2) * NB + (hp + 1) * 2 * MMN]
                    nc.scalar.activation(
                        dst,
                        y_ps[:],
                        mybir.ActivationFunctionType.Gelu,
                        bias=b2_t[:, q:q + 1],
                        scale=1.0,
                    )
                if q % 2 == 1:
                    # gpsimd SWDGE spreads paired stores over all 16 SDMA
                    # engines; the last pair goes on scalar to overlap.
                    eng = nc.scalar if q == NQ - 1 else nc.gpsimd
                    eng.dma_start(yp_d[q // 2], pair_t[:])

    nc.compile()
    nc.finalize()
    return nc


def _get_nc():
    global _NC
    if _NC is None:
        _NC = _build_module()
    return _NC


def _banded_attn(w1, w2, sparse_mask):
    """Host-side: (w1.T @ w2.T) * mask as a padded dense band, fp16.

    Mask nonzeros live in 6x6 blocks at (a + 6j + u, b + 6j + v) for shift
    pairs (a, b) in {(0,0), (64,0), (128,0), (0,64), (0,128)}; each block is
    one 6x4096x6 matmul, batched over j.  Returns [PADROWS, D] fp16 where
    row r corresponds to s = r - PADT.
    """
    PAD = 144  # covers max shift 128 + block overhang 6
    w1p = np.zeros((D, D + PAD), dtype=np.float32)
    w1p[:, :D] = w1
    w2p = np.zeros((D + PAD, D), dtype=np.float32)
    w2p[:D] = w2
    nblk = D // 6 + 1  # 683 blocks of 6 cover every in-range (s, t) block
    attn_pad = np.zeros((PADROWS, D + PAD), dtype=np.float32)
    jj = 6 * np.arange(nblk)
    for a, b in ((0, 0), (64, 0), (128, 0), (0, 64), (0, 128)):
        w1b = np.ascontiguousarray(
            w1p[:, a:a + 6 * nblk].reshape(D, nblk, 6).transpose(1, 2, 0))
        w2b = np.ascontiguousarray(
            w2p[b:b + 6 * nblk].reshape(nblk, 6, D))
        blocks = np.matmul(w1b, w2b.transpose(0, 2, 1))  # [nblk, 6 u, 6 v]
        rows = (PADT + a + jj[:, None] + np.arange(6)[None, :])  # [j, u]
        cols = (b + jj[:, None] + np.arange(6)[None, :])         # [j, v]
        attn_pad[rows[:, :, None], cols[:, None, :]] = blocks
    attn_pad = attn_pad[:, :D]
    attn_pad[:PADT] = 0.0
    attn_pad[PADT + D:] = 0.0
    attn_pad[PADT:PADT + D] *= np.asarray(sparse_mask, dtype=np.float32)
    return attn_pad.astype(np.float16)


def _band_ok(sparse_mask):
    """Every mask nonzero must sit in one of the five 6x6-block bands the
    host attn construction covers (and inside the kernel's s-window)."""
    s_idx, t_idx = np.nonzero(np.asarray(sparse_mask) != 0)
    if len(s_idx) == 0:
        return True
    da = s_idx - 6 * (t_idx // 6)   # type A: t-block-aligned
    db = t_idx - 6 * (s_idx // 6)   # type B: s-block-aligned
    allowed = np.concatenate([np.arange(k, k + 6) for k in (0, 64, 128)])
    return bool(np.all(np.isin(da, allowed) | np.isin(db, allowed)))


def prepare_in_maps(x, w1, w2, b2, sparse_mask):
    x = np.asarray(x, dtype=np.float32)
    w1 = np.asarray(w1, dtype=np.float32)
    w2 = np.asarray(w2, dtype=np.float32)
    b2 = np.asarray(b2, dtype=np.float32)

    attn16 = _banded_attn(w1, w2, sparse_mask)       # [PADROWS, D] fp16

    xT_pad = np.zeros((PADROWS, N), dtype=np.float16)
    xT_pad[PADT:PADT + D] = x.reshape(N, D).T

    in_maps = []
    for i in range(NCORES):
        it, inn = divmod(i, CN)
        t0 = it * TB
        n0 = inn * NB
        xw = np.ascontiguousarray(
            xT_pad[t0:t0 + NCH * P, n0:n0 + NB]).reshape(NCH, P, NB)
        at = np.empty((NQ, BANDCH, P, P), dtype=np.float16)
        for q in range(NQ):
            for c in range(BANDCH):
                r0 = t0 + P * (q + c)
                at[q, c] = attn16[r0:r0 + P, t0 + P * q:t0 + P * (q + 1)]
        in_maps.append({
            "xq_s": np.ascontiguousarray(
                xw[:4].transpose(1, 0, 2).reshape(P, 4 * NB)),
            "attn_s": np.ascontiguousarray(
                at.transpose(2, 0, 1, 3).reshape(P, NQ * BANDCH * P)),
            "xc4_s": xw[4],
            "xs_s": np.ascontiguousarray(xw[5:]),
            "b2c_s": np.ascontiguousarray(b2[t0:t0 + TB].reshape(NQ, P).T),
        })
    return in_maps


def assemble(results):
    out = np.empty((N, D), dtype=np.float32)
    for i in range(NCORES):
        it, inn = divmod(i, CN)
        t0 = it * TB
        n0 = inn * NB
        yp = results[i]["yp_s"]                      # [4, P, 2*NB]
        yT = np.empty((NQ, P, NB), dtype=np.float16)
        yT[0::2] = yp[:, :, :NB]
        yT[1::2] = yp[:, :, NB:]
        out[n0:n0 + NB, t0:t0 + TB] = (
            yT.transpose(2, 0, 1).reshape(NB, TB).astype(np.float32))
    return out.reshape(B, T, D)


def _reference_fallback(x, w1, w2, b2, sparse_mask):
    import jax
    import jax.numpy as jnp

    cpu = jax.devices("cpu")[0]
    with jax.default_device(cpu):
        attn = jnp.einsum("ds,td->st", jnp.asarray(w1), jnp.asarray(w2))
        attn = attn * jnp.asarray(sparse_mask)
        y = jnp.einsum("bds,st->bdt", jnp.asarray(x), attn) + jnp.asarray(b2)
        return np.asarray(jax.nn.gelu(y, approximate=False), dtype=np.float32)


def kernel(x, w1, w2, b2, sparse_mask):
    import time

    from concourse.bass_utils import run_bass_kernel_spmd

    if (np.shape(x) != (B, T, D) or np.shape(w1) != (D, D)
            or np.shape(w2) != (D, D) or np.shape(b2) != (D,)
            or np.shape(sparse_mask) != (D, D) or not _band_ok(sparse_mask)):
        return _reference_fallback(x, w1, w2, b2, sparse_mask)

    in_maps = prepare_in_maps(x, w1, w2, b2, sparse_mask)
    nc = _get_nc()
    last_err = None
    for attempt in range(3):
        try:
            res = run_bass_kernel_spmd(nc, in_maps, list(range(NCORES)))
            return assemble(res.results)
        except Exception as e:  # transient NRT/device errors: retry
            last_err = e
            time.sleep(2.0 * (attempt + 1))
    raise last_err


# revision 18
# speedup vs baseline: 1.2540x; 1.0115x over previous
"""Butterfly sparse-attention MLP kernel for 8 Trainium2 NeuronCores.

Computation (from the reference):
    attn = (w1.T @ w2.T) * sparse_mask          # [4096 s, 4096 t]
    y    = gelu(x @ attn + b2)                  # [8, 768, 4096]

sparse_mask has ~140k nonzeros in 6x6 blocks on five bands (offsets
0/+-64/+-128): attn[s, t] == 0 whenever |s - t| > 133.  Computing attn is
only ~1.2 GFLOP (0.6% of the total work), so the host computes the banded
attn during input prep via batched 6x4096x6 matmuls; the device kernel is
just the 206-GFLOP banded  y = gelu(x @ attn + b2).

Sharding: 4-way tensor-parallel over t (blocks of 1024) x 2-way data
parallel over rows n (blocks of 3072).  Each core loads an 1408-row
s-window of x^T (its 1024 t-block + 133-margin band, 11 chunks of 128),
the banded attn tiles for its block (1 MB), and streams
  yT[q] = gelu(attn[band(q)].T @ x_window + b2)     per 128-t subtile q,
contracting 4 s-chunks of 128 per subtile (band width 394 <= 512).

DMA queues dispatch ~1 descriptor (one SBUF partition row) per ~50 ns, so
every [128, W] transfer costs ~6.4 us of queue time until W reaches ~18 KB
rows (the HBM byte limit).  x travels as 24 KB-row quad-chunks, attn as one
8 KB-row transfer on the otherwise idle vector queue, y as 12 KB-row pair
stores; the final subtile is stored as four 32-partition slices on four
queues to cut the drain tail to ~2 us.  Prefetch order is gated so the
first quad (subtile 0's whole band) owns the full HBM bandwidth.
"""

import numpy as np

B, T, D = 8, 768, 4096
N = B * T              # 6144 rows of x
NCORES = 8
CT, CN = 4, 2          # t-split x n-split core grid
TB = D // CT           # 1024 t-columns per core
NB = N // CN           # 3072 x-rows per core
P = 128
NQ = TB // P           # 8 t-subtiles per core
MARGIN = 133           # band half-width of attn
NCH = 11               # s-chunks in the x window (128*11 = 1408 >= 1024+2*133)
BANDCH = 4             # s-chunks feeding one t-subtile (394-wide band)
MMN = 512              # moving-operand free-dim cap per matmul
NH = NB // MMN         # 6 n-pieces per core
PADT = MARGIN          # top padding of the s axis
PADROWS = D + P * (NCH - NQ) + PADT  # 4480: fits every core's window

_NC = None


def _build_module():
    from concourse import bacc, bass, mybir, tile

    f32 = mybir.dt.float32
    f16 = mybir.dt.float16
    PSUM = bass.MemorySpace.PSUM

    nc = bacc.Bacc("TRN2", target_bir_lowering=False, debug=False)
    xq_d = nc.declare_dram_parameter("xq_s", [P, 4 * NB], f16, isOutput=False)
    attn_d = nc.declare_dram_parameter(
        "attn_s", [P, NQ * BANDCH * P], f16, isOutput=False)
    xc4_d = nc.declare_dram_parameter("xc4_s", [P, NB], f16, isOutput=False)
    xs_d = nc.declare_dram_parameter("xs_s", [6, P, NB], f16, isOutput=False)
    b2_d = nc.declare_dram_parameter("b2c_s", [P, NQ], f32, isOutput=False)
    yp_d = nc.declare_dram_parameter("yp_s", [NQ // 2, P, 2 * NB], f16,
                                     isOutput=True)

    AQ = 2 * BANDCH * P  # attn columns for the first two subtiles

    with tile.TileContext(nc) as tc:
        with (
            tc.tile_pool(name="const", bufs=1) as cpool,
            tc.tile_pool(name="xp", bufs=1) as xp,
            tc.tile_pool(name="psB", bufs=4, space=PSUM) as psB,
            tc.tile_pool(name="yp", bufs=4) as yp,
        ):
            # Head fill (~4 MB gates the first matmul).  The sync HWDGE
            # queue is the fastest (scalar's runs ~2-3x slower), so all
            # head-critical loads go there in priority order: attn, then
            # the chunk 0-3 quad (24 KB rows), then chunks 5-10.  gpsimd
            # takes b2 + chunk 4; the scalar queue carries no loads.
            attn_t = cpool.tile([P, NQ * BANDCH * P], f16)
            nc.sync.dma_start(attn_t[:], attn_d[:])
            xq0 = xp.tile([P, 4 * NB], f16, name="xq0")
            nc.sync.dma_start(xq0[:], xq_d[:])
            b2_t = cpool.tile([P, NQ], f32)
            nc.gpsimd.dma_start(b2_t[:], b2_d[:])
            xc4 = xp.tile([P, NB], f16, name="xc4")
            nc.gpsimd.dma_start(xc4[:], xc4_d[:])
            xs = []
            for k in range(6):  # chunks 5..10 queue behind the quad on sync
                xt = xp.tile([P, NB], f16, name=f"xs{k}")
                nc.sync.dma_start(xt[:], xs_d[k])
                xs.append(xt)

            def xchunk(c):
                if c < 4:
                    return xq0[:, c * NB:(c + 1) * NB]
                if c == 4:
                    return xc4[:]
                return xs[c - 5][:]

            pair_t = None
            for q in range(NQ):
                if q % 2 == 0:
                    pair_t = yp.tile([P, 2 * NB], f16, name="y_pair",
                                     tag="y_pair")
                for hp in range(NH // 2):
                    y_ps = psB.tile([P, 2 * MMN], f32, name="y_ps", tag="y_ps")
                    for c in range(BANDCH):
                        a_sl = attn_t[:, (BANDCH * q + c) * P:
                                      (BANDCH * q + c + 1) * P]
                        xc = xchunk(q + c)
                        for hh in range(2):
                            h = 2 * hp + hh
                            nc.tensor.matmul(
                                y_ps[:, hh * MMN:(hh + 1) * MMN],
                                a_sl,
                                xc[:, h * MMN:(h + 1) * MMN],
                                start=(c == 0),
                                stop=(c == BANDCH - 1),
                            )
                    dst = pair_t[:, (q % 2) * NB + hp * 2 * MMN:
                                 (q % 2) * NB + (hp + 1) * 2 * MMN]
                    nc.scalar.activation(
                        dst,
                        y_ps[:],
                        mybir.ActivationFunctionType.Gelu,
                        bias=b2_t[:, q:q + 1],
                        scale=1.0,
                    )
                if q % 2 == 1:
                    # Pair stores ride gpsimd; the last pair goes on the
                    # fast (and by then idle) sync queue to cut the tail.
                    eng = nc.sync if q == NQ - 1 else nc.gpsimd
                    eng.dma_start(yp_d[q // 2], pair_t[:])

    nc.compile()
    nc.finalize()
    return nc


def _get_nc():
    global _NC
    if _NC is None:
        _NC = _build_module()
    return _NC


def _banded_attn(w1, w2, sparse_mask):
    """Host-side: (w1.T @ w2.T) * mask as a padded dense band, fp16.

    Mask nonzeros live in 6x6 blocks at (a + 6j + u, b + 6j + v) for shift
    pairs (a, b) in {(0,0), (64,0), (128,0), (0,64), (0,128)}; each block is
    one 6x4096x6 matmul, batched over j.  Returns [PADROWS, D] fp16 where
    row r corresponds to s = r - PADT.
    """
    PAD = 144  # covers max shift 128 + block overhang 6
    w1p = np.zeros((D, D + PAD), dtype=np.float32)
    w1p[:, :D] = w1
    w2p = np.zeros((D + PAD, D), dtype=np.float32)
    w2p[:D] = w2
    nblk = D // 6 + 1  # 683 blocks of 6 cover every in-range (s, t) block
    attn_pad = np.zeros((PADROWS, D + PAD), dtype=np.float32)
    jj = 6 * np.arange(nblk)
    for a, b in ((0, 0), (64, 0), (128, 0), (0, 64), (0, 128)):
        w1b = np.ascontiguousarray(
            w1p[:, a:a + 6 * nblk].reshape(D, nblk, 6).transpose(1, 2, 0))
        w2b = np.ascontiguousarray(
            w2p[b:b + 6 * nblk].reshape(nblk, 6, D))
        blocks = np.matmul(w1b, w2b.transpose(0, 2, 1))  # [nblk, 6 u, 6 v]
        rows = (PADT + a + jj[:, None] + np.arange(6)[None, :])  # [j, u]
        cols = (b + jj[:, None] + np.arange(6)[None, :])         # [j, v]
        attn_pad[rows[:, :, None], cols[:, None, :]] = blocks
    attn_pad = attn_pad[:, :D]
    attn_pad[:PADT] = 0.0
    attn_pad[PADT + D:] = 0.0
    attn_pad[PADT:PADT + D] *= np.asarray(sparse_mask, dtype=np.float32)
    return attn_pad.astype(np.float16)


def _band_ok(sparse_mask):
    """Every mask nonzero must sit in one of the five 6x6-block bands the
    host attn construction covers (and inside the kernel's s-window)."""
    s_idx, t_idx = np.nonzero(np.asarray(sparse_mask) != 0)
    if len(s_idx) == 0:
        return True
    da = s_idx - 6 * (t_idx // 6)   # type A: t-block-aligned
    db = t_idx - 6 * (s_idx // 6)   # type B: s-block-aligned
    allowed = np.concatenate([np.arange(k, k + 6) for k in (0, 64, 128)])
    return bool(np.all(np.isin(da, allowed) | np.isin(db, allowed)))


def prepare_in_maps(x, w1, w2, b2, sparse_mask):
    x = np.asarray(x, dtype=np.float32)
    w1 = np.asarray(w1, dtype=np.float32)
    w2 = np.asarray(w2, dtype=np.float32)
    b2 = np.asarray(b2, dtype=np.float32)

    attn16 = _banded_attn(w1, w2, sparse_mask)       # [PADROWS, D] fp16

    xT_pad = np.zeros((PADROWS, N), dtype=np.float16)
    xT_pad[PADT:PADT + D] = x.reshape(N, D).T

    in_maps = []
    for i in range(NCORES):
        it, inn = divmod(i, CN)
        t0 = it * TB
        n0 = inn * NB
        xw = np.ascontiguousarray(
            xT_pad[t0:t0 + NCH * P, n0:n0 + NB]).reshape(NCH, P, NB)
        at = np.empty((NQ, BANDCH, P, P), dtype=np.float16)
        for q in range(NQ):
            for c in range(BANDCH):
                r0 = t0 + P * (q + c)
                at[q, c] = attn16[r0:r0 + P, t0 + P * q:t0 + P * (q + 1)]
        in_maps.append({
            "xq_s": np.ascontiguousarray(
                xw[:4].transpose(1, 0, 2).reshape(P, 4 * NB)),
            "attn_s": np.ascontiguousarray(
                at.transpose(2, 0, 1, 3).reshape(P, NQ * BANDCH * P)),
            "xc4_s": xw[4],
            "xs_s": np.ascontiguousarray(xw[5:]),
            "b2c_s": np.ascontiguousarray(b2[t0:t0 + TB].reshape(NQ, P).T),
        })
    return in_maps


def assemble(results):
    out = np.empty((N, D), dtype=np.float32)
    for i in range(NCORES):
        it, inn = divmod(i, CN)
        t0 = it * TB
        n0 = inn * NB
        yp = results[i]["yp_s"]                      # [4, P, 2*NB]
        yT = np.empty((NQ, P, NB), dtype=np.float16)
        yT[0::2] = yp[:, :, :NB]
        yT[1::2] = yp[:, :, NB:]
        out[n0:n0 + NB, t0:t0 + TB] = (
            yT.transpose(2, 0, 1).reshape(NB, TB).astype(np.float32))
    return out.reshape(B, T, D)


def _reference_fallback(x, w1, w2, b2, sparse_mask):
    import jax
    import jax.numpy as jnp

    cpu = jax.devices("cpu")[0]
    with jax.default_device(cpu):
        attn = jnp.einsum("ds,td->st", jnp.asarray(w1), jnp.asarray(w2))
        attn = attn * jnp.asarray(sparse_mask)
        y = jnp.einsum("bds,st->bdt", jnp.asarray(x), attn) + jnp.asarray(b2)
        return np.asarray(jax.nn.gelu(y, approximate=False), dtype=np.float32)


def kernel(x, w1, w2, b2, sparse_mask):
    import time

    from concourse.bass_utils import run_bass_kernel_spmd

    if (np.shape(x) != (B, T, D) or np.shape(w1) != (D, D)
            or np.shape(w2) != (D, D) or np.shape(b2) != (D,)
            or np.shape(sparse_mask) != (D, D) or not _band_ok(sparse_mask)):
        return _reference_fallback(x, w1, w2, b2, sparse_mask)

    in_maps = prepare_in_maps(x, w1, w2, b2, sparse_mask)
    nc = _get_nc()
    last_err = None
    for attempt in range(3):
        try:
            res = run_bass_kernel_spmd(nc, in_maps, list(range(NCORES)))
            return assemble(res.results)
        except Exception as e:  # transient NRT/device errors: retry
            last_err = e
            time.sleep(2.0 * (attempt + 1))
    raise last_err


# revision 20
# speedup vs baseline: 1.3461x; 1.0735x over previous
"""Butterfly sparse-attention MLP kernel for 8 Trainium2 NeuronCores.

Computation (from the reference):
    attn = (w1.T @ w2.T) * sparse_mask          # [4096 s, 4096 t]
    y    = gelu(x @ attn + b2)                  # [8, 768, 4096]

sparse_mask has ~140k nonzeros in 6x6 blocks on five bands (offsets
0/+-64/+-128): attn[s, t] == 0 whenever |s - t| > 133.  Computing attn is
only ~1.2 GFLOP (0.6% of the total work), so the host computes the banded
attn during input prep via batched 6x4096x6 matmuls; the device kernel is
just the 206-GFLOP banded  y = gelu(x @ attn + b2).

Sharding: 4-way tensor-parallel over t (blocks of 1024) x 2-way data
parallel over rows n (blocks of 3072).  Each core loads an 1408-row
s-window of x^T (its 1024 t-block + 133-margin band, 11 chunks of 128),
the banded attn tiles for its block (1 MB), and streams
  yT[q] = gelu(attn[band(q)].T @ x_window + b2)     per 128-t subtile q,
contracting 4 s-chunks of 128 per subtile (band width 394 <= 512).

DMA queues dispatch ~1 descriptor (one SBUF partition row) per ~50 ns, so
every [128, W] transfer costs ~6.4 us of queue time until W reaches ~18 KB
rows (the HBM byte limit).  x travels as 24 KB-row quad-chunks, attn as one
8 KB-row transfer on the otherwise idle vector queue, y as 12 KB-row pair
stores; the final subtile is stored as four 32-partition slices on four
queues to cut the drain tail to ~2 us.  Prefetch order is gated so the
first quad (subtile 0's whole band) owns the full HBM bandwidth.
"""

import numpy as np

B, T, D = 8, 768, 4096
N = B * T              # 6144 rows of x
NCORES = 8
CT, CN = 4, 2          # t-split x n-split core grid
TB = D // CT           # 1024 t-columns per core
NB = N // CN           # 3072 x-rows per core
P = 128
NQ = TB // P           # 8 t-subtiles per core
MARGIN = 133           # band half-width of attn
NCH = 11               # s-chunks in the x window (128*11 = 1408 >= 1024+2*133)
BANDCH = 4             # s-chunks feeding one t-subtile (394-wide band)
MMN = 512              # moving-operand free-dim cap per matmul
NH = NB // MMN         # 6 n-pieces per core
PADT = MARGIN          # top padding of the s axis
PADROWS = D + P * (NCH - NQ) + PADT  # 4480: fits every core's window

_NC = None


def _build_module():
    from concourse import bacc, bass, mybir, tile

    f32 = mybir.dt.float32
    f16 = mybir.dt.float16
    PSUM = bass.MemorySpace.PSUM

    nc = bacc.Bacc("TRN2", target_bir_lowering=False, debug=False)
    attn_d = nc.declare_dram_parameter(
        "attn_s", [P, NQ * BANDCH * P], f16, isOutput=False)
    xc4_d = nc.declare_dram_parameter("xc4_s", [P, NB], f16, isOutput=False)
    xs_d = nc.declare_dram_parameter("xs_s", [10, P, NB], f16, isOutput=False)
    b2_d = nc.declare_dram_parameter("b2c_s", [P, NQ], f32, isOutput=False)
    yp_d = nc.declare_dram_parameter("yp_s", [NQ // 2, P, 2 * NB], f16,
                                     isOutput=True)

    with tile.TileContext(nc) as tc:
        with (
            tc.tile_pool(name="const", bufs=1) as cpool,
            tc.tile_pool(name="xp", bufs=1) as xp,
            tc.tile_pool(name="psB", bufs=4, space=PSUM) as psB,
            tc.tile_pool(name="yp", bufs=4) as yp,
        ):
            # Head fill (~4 MB gates subtile 0).  The sync HWDGE queue is
            # the fastest (scalar's runs ~2-3x slower), so all loads go
            # there in priority order: attn, chunks 0-3 as singles (so
            # subtile 0's accumulation passes can chase their arrival),
            # then chunks 5-10.  gpsimd takes b2 + chunk 4; the scalar
            # queue carries no loads.
            attn_t = cpool.tile([P, NQ * BANDCH * P], f16)
            nc.sync.dma_start(attn_t[:], attn_d[:])
            xs = []
            for k in range(10):  # chunks 0,1,2,3,5,6,...,10
                xt = xp.tile([P, NB], f16, name=f"xs{k}")
                nc.sync.dma_start(xt[:], xs_d[k])
                xs.append(xt)
            b2_t = cpool.tile([P, NQ], f32)
            nc.gpsimd.dma_start(b2_t[:], b2_d[:])
            xc4 = xp.tile([P, NB], f16, name="xc4")
            nc.gpsimd.dma_start(xc4[:], xc4_d[:])

            def xchunk(c):
                if c < 4:
                    return xs[c][:]
                if c == 4:
                    return xc4[:]
                return xs[c - 1][:]

            def a_slice(q, c):
                return attn_t[:, (BANDCH * q + c) * P:(BANDCH * q + c + 1) * P]

            pair_t = yp.tile([P, 2 * NB], f16, name="y_pair", tag="y_pair")

            # Subtile 0: c-outer so each accumulation pass starts as soon
            # as its chunk lands; PSUM holds all three n-piece tiles.
            pss0 = [psB.tile([P, 2 * MMN], f32, name="y_ps", tag="y_ps")
                    for _ in range(NH // 2)]
            for c in range(BANDCH):
                for hp in range(NH // 2):
                    for hh in range(2):
                        h = 2 * hp + hh
                        nc.tensor.matmul(
                            pss0[hp][:, hh * MMN:(hh + 1) * MMN],
                            a_slice(0, c),
                            xchunk(c)[:, h * MMN:(h + 1) * MMN],
                            start=(c == 0),
                            stop=(c == BANDCH - 1),
                        )
            for hp in range(NH // 2):
                nc.scalar.activation(
                    pair_t[:, hp * 2 * MMN:(hp + 1) * 2 * MMN],
                    pss0[hp][:],
                    mybir.ActivationFunctionType.Gelu,
                    bias=b2_t[:, 0:1],
                    scale=1.0,
                )

            for q in range(1, NQ):
                if q % 2 == 0:
                    pair_t = yp.tile([P, 2 * NB], f16, name="y_pair",
                                     tag="y_pair")
                for hp in range(NH // 2):
                    y_ps = psB.tile([P, 2 * MMN], f32, name="y_ps", tag="y_ps")
                    for c in range(BANDCH):
                        a_sl = a_slice(q, c)
                        xc = xchunk(q + c)
                        for hh in range(2):
                            h = 2 * hp + hh
                            nc.tensor.matmul(
                                y_ps[:, hh * MMN:(hh + 1) * MMN],
                                a_sl,
                                xc[:, h * MMN:(h + 1) * MMN],
                                start=(c == 0),
                                stop=(c == BANDCH - 1),
                            )
                    dst = pair_t[:, (q % 2) * NB + hp * 2 * MMN:
                                 (q % 2) * NB + (hp + 1) * 2 * MMN]
                    nc.scalar.activation(
                        dst,
                        y_ps[:],
                        mybir.ActivationFunctionType.Gelu,
                        bias=b2_t[:, q:q + 1],
                        scale=1.0,
                    )
                if q % 2 == 1:
                    # Pair stores ride gpsimd; the last pair goes on the
                    # fast (and by then idle) sync queue to cut the tail.
                    eng = nc.sync if q == NQ - 1 else nc.gpsimd
                    eng.dma_start(yp_d[q // 2], pair_t[:])

    nc.compile()
    nc.finalize()
    return nc


def _get_nc():
    global _NC
    if _NC is None:
        _NC = _build_module()
    return _NC


def _banded_attn(w1, w2, sparse_mask):
    """Host-side: (w1.T @ w2.T) * mask as a padded dense band, fp16.

    Mask nonzeros live in 6x6 blocks at (a + 6j + u, b + 6j + v) for shift
    pairs (a, b) in {(0,0), (64,0), (128,0), (0,64), (0,128)}; each block is
    one 6x4096x6 matmul, batched over j.  Returns [PADROWS, D] fp16 where
    row r corresponds to s = r - PADT.
    """
    PAD = 144  # covers max shift 128 + block overhang 6
    w1p = np.zeros((D, D + PAD), dtype=np.float32)
    w1p[:, :D] = w1
    w2p = np.zeros((D + PAD, D), dtype=np.float32)
    w2p[:D] = w2
    nblk = D // 6 + 1  # 683 blocks of 6 cover every in-range (s, t) block
    attn_pad = np.zeros((PADROWS, D + PAD), dtype=np.float32)
    jj = 6 * np.arange(nblk)
    for a, b in ((0, 0), (64, 0), (128, 0), (0, 64), (0, 128)):
        w1b = np.ascontiguousarray(
            w1p[:, a:a + 6 * nblk].reshape(D, nblk, 6).transpose(1, 2, 0))
        w2b = np.ascontiguousarray(
            w2p[b:b + 6 * nblk].reshape(nblk, 6, D))
        blocks = np.matmul(w1b, w2b.transpose(0, 2, 1))  # [nblk, 6 u, 6 v]
        rows = (PADT + a + jj[:, None] + np.arange(6)[None, :])  # [j, u]
        cols = (b + jj[:, None] + np.arange(6)[None, :])         # [j, v]
        attn_pad[rows[:, :, None], cols[:, None, :]] = blocks
    attn_pad = attn_pad[:, :D]
    attn_pad[:PADT] = 0.0
    attn_pad[PADT + D:] = 0.0
    attn_pad[PADT:PADT + D] *= np.asarray(sparse_mask, dtype=np.float32)
    return attn_pad.astype(np.float16)


def _band_ok(sparse_mask):
    """Every mask nonzero must sit in one of the five 6x6-block bands the
    host attn construction covers (and inside the kernel's s-window)."""
    s_idx, t_idx = np.nonzero(np.asarray(sparse_mask) != 0)
    if len(s_idx) == 0:
        return True
    da = s_idx - 6 * (t_idx // 6)   # type A: t-block-aligned
    db = t_idx - 6 * (s_idx // 6)   # type B: s-block-aligned
    allowed = np.concatenate([np.arange(k, k + 6) for k in (0, 64, 128)])
    return bool(np.all(np.isin(da, allowed) | np.isin(db, allowed)))


def prepare_in_maps(x, w1, w2, b2, sparse_mask):
    x = np.asarray(x, dtype=np.float32)
    w1 = np.asarray(w1, dtype=np.float32)
    w2 = np.asarray(w2, dtype=np.float32)
    b2 = np.asarray(b2, dtype=np.float32)

    attn16 = _banded_attn(w1, w2, sparse_mask)       # [PADROWS, D] fp16

    xT_pad = np.zeros((PADROWS, N), dtype=np.float16)
    xT_pad[PADT:PADT + D] = x.reshape(N, D).T

    in_maps = []
    for i in range(NCORES):
        it, inn = divmod(i, CN)
        t0 = it * TB
        n0 = inn * NB
        xw = np.ascontiguousarray(
            xT_pad[t0:t0 + NCH * P, n0:n0 + NB]).reshape(NCH, P, NB)
        at = np.empty((NQ, BANDCH, P, P), dtype=np.float16)
        for q in range(NQ):
            for c in range(BANDCH):
                r0 = t0 + P * (q + c)
                at[q, c] = attn16[r0:r0 + P, t0 + P * q:t0 + P * (q + 1)]
        in_maps.append({
            "attn_s": np.ascontiguousarray(
                at.transpose(2, 0, 1, 3).reshape(P, NQ * BANDCH * P)),
            "xc4_s": xw[4],
            "xs_s": np.ascontiguousarray(
                np.concatenate([xw[:4], xw[5:]], axis=0)),
            "b2c_s": np.ascontiguousarray(b2[t0:t0 + TB].reshape(NQ, P).T),
        })
    return in_maps


def assemble(results):
    out = np.empty((N, D), dtype=np.float32)
    for i in range(NCORES):
        it, inn = divmod(i, CN)
        t0 = it * TB
        n0 = inn * NB
        yp = results[i]["yp_s"]                      # [4, P, 2*NB]
        yT = np.empty((NQ, P, NB), dtype=np.float16)
        yT[0::2] = yp[:, :, :NB]
        yT[1::2] = yp[:, :, NB:]
        out[n0:n0 + NB, t0:t0 + TB] = (
            yT.transpose(2, 0, 1).reshape(NB, TB).astype(np.float32))
    return out.reshape(B, T, D)


def _reference_fallback(x, w1, w2, b2, sparse_mask):
    import jax
    import jax.numpy as jnp

    cpu = jax.devices("cpu")[0]
    with jax.default_device(cpu):
        attn = jnp.einsum("ds,td->st", jnp.asarray(w1), jnp.asarray(w2))
        attn = attn * jnp.asarray(sparse_mask)
        y = jnp.einsum("bds,st->bdt", jnp.asarray(x), attn) + jnp.asarray(b2)
        return np.asarray(jax.nn.gelu(y, approximate=False), dtype=np.float32)


def kernel(x, w1, w2, b2, sparse_mask):
    import time

    from concourse.bass_utils import run_bass_kernel_spmd

    if (np.shape(x) != (B, T, D) or np.shape(w1) != (D, D)
            or np.shape(w2) != (D, D) or np.shape(b2) != (D,)
            or np.shape(sparse_mask) != (D, D) or not _band_ok(sparse_mask)):
        return _reference_fallback(x, w1, w2, b2, sparse_mask)

    in_maps = prepare_in_maps(x, w1, w2, b2, sparse_mask)
    nc = _get_nc()
    last_err = None
    for attempt in range(3):
        try:
            res = run_bass_kernel_spmd(nc, in_maps, list(range(NCORES)))
            return assemble(res.results)
        except Exception as e:  # transient NRT/device errors: retry
            last_err = e
            time.sleep(2.0 * (attempt + 1))
    raise last_err
